# revision 32
# baseline (speedup 1.0000x reference)
"""Coordinate-descent (alternating Gauss-Seidel) kernel for Trainium2, v5.

B=4 factorizations x ~ u @ v^T, M=N=4096, R=32.
The per-column GS sweep is algebraically a triangular solve:
    u_new = (a + eps - u @ B_sl) @ M^{-1},   M = diag(B)+eps + triu(B,1)
with B = v^T v.  M^{-1} is applied exactly via the nilpotent factorization
    (I+W)^{-1} = (I-W)(I+W^2)(I+W^4)(I+W^8)(I+W^16),  W = triu(B,1) D'^{-1}
so each half-step is a handful of PE matmuls instead of a 32-step
vector-engine recurrence.  All work stays in transposed [R, m] space.

The u-side Grams b1 = v^T v and their W-chains depend only on v, so they are
hoisted to kernel start (also serving as HAM warm-up).  Phase-2 partials
(a2T = u_new^T x, b2) use 4-way col-tiled matmuls and one fused per-batch
ReduceScatter; the v-solve of batch b is emitted two batches later so no
engine queue ever stalls on a collective.
"""

import os
from contextlib import ExitStack

import numpy as np

import concourse.bass as bass
import concourse.tile as tile
from concourse import bacc, mybir
from concourse.bass import ds
from concourse.bass_utils import run_bass_kernel_spmd
from concourse.masks import make_identity, make_lower_triangular

B, M, N, R = 4, 4096, 4096, 32
NCORES = 8
MS = M // NCORES          # 512 rows per core per batch
MC = MS // 128            # 4 m-chunks of 128
NG = N // 512             # 8 n-groups of 512
NCH = N // 128            # 32 n-chunks of 128
EPS = 1e-8
F32 = mybir.dt.float32
F32R = mybir.dt.float32r
BF16 = mybir.dt.bfloat16
ALU = mybir.AluOpType
AX = mybir.AxisListType

_CACHE = {}
LAST_RESULT = None


def _gram_prep(nc, smp, pwp, punp, consts, b_sb, tg):
    """Precompute inv_p, B_sl and the W-chain from a Gram matrix (SBUF)."""
    ident32_r, masksl_r, eye_r = consts

    bd = smp.tile([R, R], F32R, tag=f"bd{tg}", name="bd", bufs=1)
    nc.vector.tensor_tensor(out=bd[:], in0=b_sb[:], in1=eye_r, op=ALU.mult)
    d_p = smp.tile([R, 1], F32, tag=f"dp{tg}", name="d_p", bufs=1)
    inv_p = smp.tile([R, 1], F32, tag=f"ip{tg}", name="inv_p", bufs=1)
    nc.vector.tensor_reduce(d_p[:], bd[:], axis=AX.X, op=ALU.add)
    nc.vector.tensor_scalar_add(inv_p[:], d_p[:], EPS)
    nc.vector.reciprocal(inv_p[:], inv_p[:])
    invb = bass.AP(inv_p[:].tensor, inv_p[:].offset, [inv_p[:].ap[0], [0, R]])

    bsl = smp.tile([R, R], F32R, tag=f"bsl{tg}", name="bsl", bufs=1)
    nc.vector.tensor_tensor(out=bsl[:], in0=b_sb[:], in1=masksl_r,
                            op=ALU.mult)
    nbsl = smp.tile([R, R], F32R, tag=f"nbsl{tg}", name="nbsl", bufs=1)
    nc.vector.tensor_scalar_mul(nbsl[:], bsl[:], -1.0)
    vw = smp.tile([R, R], F32R, tag=f"vw{tg}", name="vw", bufs=1)  # V = W^T
    nc.vector.tensor_tensor(out=vw[:], in0=bsl[:], in1=invb, op=ALU.mult)

    # transpose W^T -> W (f32r out must live in the f32r pun slot)
    pwt = punp.tile([128, MC, R], F32R, tag="pun", name="pwt")
    nc.tensor.transpose(pwt[:R, 0, :], vw[:], ident32_r)
    w1 = smp.tile([R, R], F32R, tag=f"w1{tg}", name="w1", bufs=1)
    nc.scalar.copy(w1[:], pwt[:R, 0, :])

    def _mm_small(lhsT, rhs, tagn):
        p = pwp.tile([R, R], F32, tag="pw", name="pmm")
        nc.tensor.matmul(p[:], lhsT=lhsT[:], rhs=rhs[:], start=True,
                         stop=True)
        s = smp.tile([R, R], F32R, tag=f"{tagn}{tg}", name=tagn, bufs=1)
        nc.scalar.copy(s[:], p[:])
        return s

    w2 = _mm_small(vw, w1, "w2")     # W^T.T @ W = W@W
    w2t = _mm_small(w1, vw, "w2t")   # W.T @ W^T = (W@W)^T
    w4 = _mm_small(w2t, w2, "w4")
    w4t = _mm_small(w2, w2t, "w4t")
    w8 = _mm_small(w4t, w4, "w8")
    w8t = _mm_small(w4, w4t, "w8t")
    w16 = _mm_small(w8t, w8, "w16")
    return {"bsl": bsl, "nbsl": nbsl, "inv_p": inv_p,
            "chain": (w1, w2, w4, w8, w16)}


def _apply_solve(nc, smp, zsb, punp, zps, consts, prep, ams_ap, out32,
                 outb16, tg):
    """z = P^T D'^{-1} (ams_ap = a^T - s^T + 0); back-transpose.

    ams_ap: PSUM AP already holding a^T - B_sl^T xT (s-MM fused upstream).
    """
    ident32_r, _, _ = consts
    inv_p = prep["inv_p"]

    z = zsb.tile([R, MS], F32R, tag=f"z{tg}", name="z0")
    nc.vector.tensor_scalar(out=z[:], in0=ams_ap, scalar1=EPS,
                            scalar2=inv_p[:], op0=ALU.add, op1=ALU.mult)

    H = MS // 2
    for wk, sign in zip(prep["chain"], (-1.0, 1.0, 1.0, 1.0, 1.0)):
        pz = zps("z")
        zn = zsb.tile([R, MS], F32R, tag=f"z{tg}", name="zn")
        for h in range(2):
            sl = slice(h * H, (h + 1) * H)
            nc.tensor.matmul(pz[:, sl], lhsT=wk[:], rhs=z[:, sl],
                             start=True, stop=True, skip_group_check=True)
            nc.vector.scalar_tensor_tensor(
                out=zn[:, sl], in0=pz[:, sl], scalar=sign, in1=z[:, sl],
                op0=ALU.mult, op1=ALU.add)
        z = zn

    pun = punp.tile([128, MC, R], F32R, tag="pun", name="pun")
    for i in range(MC):
        nc.tensor.transpose(pun[:, i], z[:, i * 128:(i + 1) * 128],
                            ident32_r)
    nc.scalar.copy(out32[:], pun[:])
    if outb16 is not None:
        nc.vector.tensor_copy(outb16[:], pun[:])


def _build():
    nc = bacc.Bacc("TRN2", target_bir_lowering=False, debug=False,
                   num_devices=NCORES)

    x_my = nc.dram_tensor("x_my", [B, MS, N], F32, kind="ExternalInput").ap()
    u_my = nc.dram_tensor("u_my", [B, MS, R], F32, kind="ExternalInput").ap()
    v_full = nc.dram_tensor("v_full", [B, N, R], F32,
                            kind="ExternalInput").ap()
    v_my = nc.dram_tensor("v_my", [B, MS, R], F32, kind="ExternalInput").ap()
    u_out = nc.dram_tensor("u_out", [B, MS, R], F32,
                           kind="ExternalOutput").ap()
    v_out = nc.dram_tensor("v_out", [B, MS, R], F32,
                           kind="ExternalOutput").ap()

    rs_ins = [nc.dram_tensor(f"rs_in_{b}", [NCORES * R, 512 + R], F32)
              for b in range(B)]
    rs_outs = [nc.dram_tensor(f"rs_out_{b}", [R, 512 + R], F32)
               for b in range(B)]

    with tile.TileContext(nc) as tc, ExitStack() as ctx:
        const = ctx.enter_context(tc.tile_pool(name="const", bufs=1))
        xbp = ctx.enter_context(tc.tile_pool(name="xbp", bufs=1))
        xgp = ctx.enter_context(tc.tile_pool(name="xgp", bufs=3))
        xtp = ctx.enter_context(tc.tile_pool(name="xtp", bufs=4))
        vp = ctx.enter_context(tc.tile_pool(name="vp", bufs=2))
        smp = ctx.enter_context(tc.tile_pool(name="smp", bufs=2))
        zsb = ctx.enter_context(tc.tile_pool(name="zsb", bufs=3))
        a2sp = ctx.enter_context(tc.tile_pool(name="a2sp", bufs=2))
        # PSUM banks: ppt 3 + pa1 1 + pzu 1 + pw 1 + pun 1 + pa2 1 = 8
        ppt = ctx.enter_context(tc.tile_pool(name="ppt", bufs=3,
                                             space="PSUM"))
        pa1p = ctx.enter_context(tc.tile_pool(name="pa1", bufs=1,
                                              space="PSUM"))
        pzup = ctx.enter_context(tc.tile_pool(name="pzu", bufs=1,
                                              space="PSUM"))
        pwp = ctx.enter_context(tc.tile_pool(name="pw", bufs=1,
                                             space="PSUM"))
        punp = ctx.enter_context(tc.tile_pool(name="pun", bufs=1,
                                              space="PSUM"))
        pa2p = ctx.enter_context(tc.tile_pool(name="pa2", bufs=1,
                                              space="PSUM"))

        ident128_b = const.tile([128, 128], BF16)
        make_identity(nc, ident128_b)
        ident128_f = const.tile([128, 128], F32)
        make_identity(nc, ident128_f)
        ident32_f = const.tile([R, R], F32)
        make_identity(nc, ident32_f)
        masksl_f = const.tile([R, R], F32)
        make_lower_triangular(nc, masksl_f, val=1.0, diag=False)
        ident32_r = const.tile([R, R], F32R)
        nc.vector.tensor_copy(ident32_r[:], ident32_f[:])
        masksl_r = const.tile([R, R], F32R)
        nc.vector.tensor_copy(masksl_r[:], masksl_f[:])
        consts = (ident32_r[:], masksl_r[:], ident32_r[:])

        def zps(nm):
            return pzup.tile([R, MS], F32, tag="zu", name=nm)

        def zps_v(nm):
            return pa2p.tile([128, MS], F32, tag="pa2", name=nm)[:R, :]

        # ---------- hoisted: v loads + casts; preps spread over phase1(0) --
        vbs = []
        u_preps = {}
        for b in range(B):
            v32 = vp.tile([128, NCH, R], F32, tag="v32", name="v32",
                          bufs=4)
            nc.sync.dma_start(v32[:],
                              v_full[b].rearrange("(c p) r -> p c r", p=128))
            vb = vp.tile([128, NCH, R], BF16, tag=f"vb{b}", name="vb",
                         bufs=1)
            nc.vector.tensor_copy(vb[:], v32[:])
            vbs.append(vb)

        def emit_prep(b):
            pb1 = zps("pb1")
            for j in range(NCH):
                nc.tensor.matmul(pb1[:, :R], lhsT=vbs[b][:, j, :],
                                 rhs=vbs[b][:, j, :], start=(j == 0),
                                 stop=(j == NCH - 1), skip_group_check=True)
            b1_sb = smp.tile([R, R], F32R, tag=f"b1s{b}", name="b1_sb",
                             bufs=1)
            nc.scalar.copy(b1_sb[:], pb1[:, :R])
            u_preps[b] = _gram_prep(nc, smp, pwp, punp, consts, b1_sb,
                                    f"u{b}")

        state = {}

        def emit_solve_v(b):
            st = state[b]
            a2t = smp.tile([R, MS], F32R, tag="a2t", name="a2t")
            nc.sync.dma_start(a2t[:],
                              rs_outs[b].ap()[:, 0:512].bitcast(F32R))
            b2_sb = smp.tile([R, R], F32R, tag="b2s", name="b2_sb")
            nc.sync.dma_start(b2_sb[:],
                              rs_outs[b].ap()[:, 512:512 + R].bitcast(F32R))
            prep = _gram_prep(nc, smp, pwp, punp, consts, b2_sb, f"v{b}")
            vn32 = smp.tile([128, MC, R], F32, tag="vn32", name="vn32")
            ps = zps_v("s")
            nc.tensor.matmul(ps[:], lhsT=prep["nbsl"][:], rhs=st["vT"][:],
                             start=True, stop=True)
            ams = smp.tile([R, MS], F32R, tag="at2", name="ams")
            nc.vector.tensor_tensor(out=ams[:], in0=ps[:], in1=a2t[:],
                                    op=ALU.add)
            _apply_solve(nc, smp, zsb, punp, zps_v, consts, prep,
                         ams[:], vn32, None, "v")
            nc.sync.dma_start(v_out[b].rearrange("(i p) r -> p i r", p=128),
                              vn32[:])

        p2state = {}

        def emit_phase2_part(bp, part):
            # one i-step quartet of col-tiled a2T MMs for batch bp
            gp, i = part // MC, part % MC
            unb_p, xb_p = p2state[bp]
            if i == 0 and f"pa2_{bp}_{gp}" not in p2state:
                p2state[f"pa2_{bp}_{gp}"] = pa2p.tile(
                    [128, MS], F32, tag="pa2", name="pa2")
            pa2 = p2state[f"pa2_{bp}_{gp}"]
            for p in range(4):
                g2 = gp * 4 + p
                nc.tensor.matmul(
                    pa2[p * R:(p + 1) * R, :], lhsT=unb_p[:, i, :],
                    rhs=xb_p[:, i, g2 * 512:(g2 + 1) * 512],
                    start=(i == 0), stop=(i == MC - 1),
                    tile_position=(0, p * R), skip_group_check=True)
            if i == MC - 1:
                a2st = a2sp.tile([128, MS], F32, tag="a2st", name="a2st")
                nc.scalar.copy(a2st[:], pa2[:])
                nc.sync.dma_start(
                    rs_ins[bp].ap()[ds(gp * 4 * R, 4 * R), 0:512], a2st[:])

        def finish_phase2(bp):
            unb_p, _ = p2state[bp]
            pb2 = pwp.tile([R, R], F32, tag="pw", name="pb2")
            for i in range(MC):
                nc.tensor.matmul(pb2[:], lhsT=unb_p[:, i, :],
                                 rhs=unb_p[:, i, :], start=(i == 0),
                                 stop=(i == MC - 1), skip_group_check=True)
            b2st = a2sp.tile([R, R], F32, tag="b2st", name="b2st")
            nc.scalar.copy(b2st[:], pb2[:])
            for c in range(NCORES):
                nc.sync.dma_start(
                    rs_ins[bp].ap()[ds(c * R, R), 512:512 + R], b2st[:])
            nc.gpsimd.collective_compute(
                "ReduceScatter", ALU.add,
                replica_groups=[list(range(NCORES))],
                ins=[rs_ins[bp].ap()], outs=[rs_outs[bp].ap()])

        pref = {}
        NPREF = 3
        for b in range(B):
            bi = b % 2
            # ---------------- per-batch loads + uT/vT ----------------
            u32 = vp.tile([128, MC, R], F32, tag="u32", name="u32")
            nc.sync.dma_start(u32[:],
                              u_my[b].rearrange("(i p) r -> p i r", p=128))
            vm32 = vp.tile([128, MC, R], F32, tag="vm32", name="vm32")
            nc.sync.dma_start(vm32[:],
                              v_my[b].rearrange("(i p) r -> p i r", p=128))

            put = zps("put")
            for i in range(MC):
                nc.tensor.transpose(put[:, i * 128:(i + 1) * 128],
                                    u32[:, i, :], ident128_f[:])
            uT = smp.tile([R, MS], F32R, tag="uT", name="uT")
            nc.scalar.copy(uT[:], put[:])
            pvt = zps("pvt")
            for i in range(MC):
                nc.tensor.transpose(pvt[:, i * 128:(i + 1) * 128],
                                    vm32[:, i, :], ident128_f[:])
            vT = smp.tile([R, MS], F32R, tag="vT", name="vT", bufs=3)
            nc.scalar.copy(vT[:], pvt[:])

            # ---------------- phase 1: stream x ----------------
            if b in pref:
                xb_t = pref.pop(b)
            else:
                xb_t = xbp.tile([128, MC, N], BF16, tag=f"xb{bi}",
                                name="xb")
            pa1 = pa1p.tile([R, MS], F32, tag="pa1", name="pa1")
            x_re = x_my[b].rearrange("(i p) n -> p i n", p=128)
            for g in range(NG):
                if not (b > 0 and g < NPREF):
                    xg = xgp.tile([128, MC, 512], F32, tag="xg", name="xg")
                    nc.sync.dma_start(xg[:],
                                      x_re[:, :, g * 512:(g + 1) * 512])
                    nc.vector.tensor_copy(
                        xb_t[:, :, g * 512:(g + 1) * 512], xg[:])
                for j2 in range(4):
                    j = 4 * g + j2
                    pt = ppt.tile([128, MC, 128], BF16, tag="pt", name="pt")
                    for i in range(MC):
                        nc.tensor.transpose(
                            pt[:, i], xb_t[:, i, j * 128:(j + 1) * 128],
                            ident128_b[:])
                    xt = xtp.tile([128, MC, 128], BF16, tag="xt", name="xt")
                    nc.scalar.copy(xt[:], pt[:])
                    nc.tensor.matmul(pa1[:], lhsT=vbs[b][:, j, :],
                                     rhs=xt.rearrange("p a b -> p (a b)"),
                                     start=(j == 0), stop=False,
                                     skip_group_check=True)
                if b == 0 and g < B:
                    emit_prep(g)
                if b >= 1 and g < 4:
                    emit_phase2_part(b - 1, 2 * g)
                    emit_phase2_part(b - 1, 2 * g + 1)
            if b >= 1:
                finish_phase2(b - 1)
                if b >= 3:
                    emit_solve_v(b - 3)

            # prefetch next batch's first groups so the x stream never stalls
            if b + 1 < B:
                xb_n = xbp.tile([128, MC, N], BF16, tag=f"xb{(b + 1) % 2}",
                                name="xbn")
                x_re_n = x_my[b + 1].rearrange("(i p) n -> p i n", p=128)
                for g in range(NPREF):
                    xg = xgp.tile([128, MC, 512], F32, tag="xg", name="xg")
                    nc.sync.dma_start(
                        xg[:], x_re_n[:, :, g * 512:(g + 1) * 512])
                    nc.vector.tensor_copy(
                        xb_n[:, :, g * 512:(g + 1) * 512], xg[:])
                pref[b + 1] = xb_n

            # ---------------- u solve (s-MM fused into pa1 group) --------
            nc.tensor.matmul(pa1[:], lhsT=u_preps[b]["nbsl"][:], rhs=uT[:],
                             start=False, stop=True, skip_group_check=True)
            un32 = smp.tile([128, MC, R], F32, tag="un32", name="un32")
            unb = smp.tile([128, MC, R], BF16, tag="unb", name="unb")
            _apply_solve(nc, smp, zsb, punp, zps, consts, u_preps[b],
                         pa1[:], un32, unb, "u")
            nc.sync.dma_start(u_out[b].rearrange("(i p) r -> p i r", p=128),
                              un32[:])

            p2state[b] = (unb, xb_t)
            state[b] = {"vT": vT}

        # last batch: dense phase 2 + RS, then remaining v-solves
        # b2 first so its replicate DMAs overlap the pack matmuls
        unb_t, _ = p2state[B - 1]
        pb2t = pwp.tile([R, R], F32, tag="pw", name="pb2t")
        for i in range(MC):
            nc.tensor.matmul(pb2t[:], lhsT=unb_t[:, i, :],
                             rhs=unb_t[:, i, :], start=(i == 0),
                             stop=(i == MC - 1), skip_group_check=True)
        b2stt = a2sp.tile([R, R], F32, tag="b2st", name="b2stt")
        nc.scalar.copy(b2stt[:], pb2t[:])
        for c in range(NCORES):
            nc.sync.dma_start(
                rs_ins[B - 1].ap()[ds(c * R, R), 512:512 + R], b2stt[:])
        # second pack borrows the zu bank so the two packs don't serialize
        p2state[f"pa2_{B - 1}_1"] = pzup.tile([128, MS], F32, tag="zu",
                                              name="pa2z")
        for part in range(NG):
            emit_phase2_part(B - 1, part)
        nc.gpsimd.collective_compute(
            "ReduceScatter", ALU.add,
            replica_groups=[list(range(NCORES))],
            ins=[rs_ins[B - 1].ap()], outs=[rs_outs[B - 1].ap()])
        emit_solve_v(B - 3)
        emit_solve_v(B - 2)
        emit_solve_v(B - 1)

    nc.compile()
    return nc


def kernel(x, u, v):
    global LAST_RESULT
    if "nc" not in _CACHE:
        _CACHE["nc"] = _build()
    nc = _CACHE["nc"]

    x = np.ascontiguousarray(x, dtype=np.float32)
    u = np.ascontiguousarray(u, dtype=np.float32)
    v = np.ascontiguousarray(v, dtype=np.float32)

    in_maps = []
    for c in range(NCORES):
        sl = slice(c * MS, (c + 1) * MS)
        in_maps.append({
            "x_my": np.ascontiguousarray(x[:, sl, :]),
            "u_my": np.ascontiguousarray(u[:, sl, :]),
            "v_full": v,
            "v_my": np.ascontiguousarray(v[:, sl, :]),
        })

    res = run_bass_kernel_spmd(nc, in_maps, list(range(NCORES)),
                               trace=os.environ.get("KBENCH_TRACE") == "1")
    LAST_RESULT = res
    u_new = np.concatenate([res.results[c]["u_out"] for c in range(NCORES)],
                           axis=1)
    v_new = np.concatenate([res.results[c]["v_out"] for c in range(NCORES)],
                           axis=1)
    return (u_new, v_new)


# revision 33
# speedup vs baseline: 1.0220x; 1.0220x over previous
"""Coordinate-descent (alternating Gauss-Seidel) kernel for Trainium2, v5.

B=4 factorizations x ~ u @ v^T, M=N=4096, R=32.
The per-column GS sweep is algebraically a triangular solve:
    u_new = (a + eps - u @ B_sl) @ M^{-1},   M = diag(B)+eps + triu(B,1)
with B = v^T v.  M^{-1} is applied exactly via the nilpotent factorization
    (I+W)^{-1} = (I-W)(I+W^2)(I+W^4)(I+W^8)(I+W^16),  W = triu(B,1) D'^{-1}
so each half-step is a handful of PE matmuls instead of a 32-step
vector-engine recurrence.  All work stays in transposed [R, m] space.

The u-side Grams b1 = v^T v and their W-chains depend only on v, so they are
hoisted to kernel start (also serving as HAM warm-up).  Phase-2 partials
(a2T = u_new^T x, b2) use 4-way col-tiled matmuls and one fused per-batch
ReduceScatter; the v-solve of batch b is emitted two batches later so no
engine queue ever stalls on a collective.
"""

import os
from contextlib import ExitStack

import numpy as np

import concourse.bass as bass
import concourse.tile as tile
from concourse import bacc, mybir
from concourse.bass import ds
from concourse.bass_utils import run_bass_kernel_spmd
from concourse.masks import make_identity, make_lower_triangular

B, M, N, R = 4, 4096, 4096, 32
NCORES = 8
MS = M // NCORES          # 512 rows per core per batch
MC = MS // 128            # 4 m-chunks of 128
NG = N // 512             # 8 n-groups of 512
NCH = N // 128            # 32 n-chunks of 128
EPS = 1e-8
F32 = mybir.dt.float32
F32R = mybir.dt.float32r
BF16 = mybir.dt.bfloat16
ALU = mybir.AluOpType
AX = mybir.AxisListType

_CACHE = {}
LAST_RESULT = None


def _gram_prep(nc, smp, pwp, punp, consts, b_sb, tg):
    """Precompute inv_p, B_sl and the W-chain from a Gram matrix (SBUF)."""
    ident32_r, masksl_r, eye_r = consts

    bd = smp.tile([R, R], F32R, tag=f"bd{tg}", name="bd", bufs=1)
    nc.vector.tensor_tensor(out=bd[:], in0=b_sb[:], in1=eye_r, op=ALU.mult)
    d_p = smp.tile([R, 1], F32, tag=f"dp{tg}", name="d_p", bufs=1)
    inv_p = smp.tile([R, 1], F32, tag=f"ip{tg}", name="inv_p", bufs=1)
    nc.vector.tensor_reduce(d_p[:], bd[:], axis=AX.X, op=ALU.add)
    nc.vector.tensor_scalar_add(inv_p[:], d_p[:], EPS)
    nc.vector.reciprocal(inv_p[:], inv_p[:])
    invb = bass.AP(inv_p[:].tensor, inv_p[:].offset, [inv_p[:].ap[0], [0, R]])

    bsl = smp.tile([R, R], F32R, tag=f"bsl{tg}", name="bsl", bufs=1)
    nc.vector.tensor_tensor(out=bsl[:], in0=b_sb[:], in1=masksl_r,
                            op=ALU.mult)
    nbsl = smp.tile([R, R], F32R, tag=f"nbsl{tg}", name="nbsl", bufs=1)
    nc.vector.tensor_scalar_mul(nbsl[:], bsl[:], -1.0)
    vw = smp.tile([R, R], F32R, tag=f"vw{tg}", name="vw", bufs=1)  # V = W^T
    nc.vector.tensor_tensor(out=vw[:], in0=bsl[:], in1=invb, op=ALU.mult)

    # transpose W^T -> W (f32r out must live in the f32r pun slot)
    pwt = punp.tile([128, MC, R], F32R, tag="pun", name="pwt")
    nc.tensor.transpose(pwt[:R, 0, :], vw[:], ident32_r)
    w1 = smp.tile([R, R], F32R, tag=f"w1{tg}", name="w1", bufs=1)
    nc.scalar.copy(w1[:], pwt[:R, 0, :])

    def _mm_small(lhsT, rhs, tagn):
        p = pwp.tile([R, R], F32, tag="pw", name="pmm")
        nc.tensor.matmul(p[:], lhsT=lhsT[:], rhs=rhs[:], start=True,
                         stop=True)
        s = smp.tile([R, R], F32R, tag=f"{tagn}{tg}", name=tagn, bufs=1)
        nc.scalar.copy(s[:], p[:])
        return s

    w2 = _mm_small(vw, w1, "w2")     # W^T.T @ W = W@W
    w2t = _mm_small(w1, vw, "w2t")   # W.T @ W^T = (W@W)^T
    w4 = _mm_small(w2t, w2, "w4")
    w4t = _mm_small(w2, w2t, "w4t")
    w8 = _mm_small(w4t, w4, "w8")
    w8t = _mm_small(w4, w4t, "w8t")
    w16 = _mm_small(w8t, w8, "w16")
    return {"bsl": bsl, "nbsl": nbsl, "inv_p": inv_p,
            "chain": (w1, w2, w4, w8, w16)}


def _apply_solve(nc, smp, zsb, punp, zps, consts, prep, ams_ap, out32,
                 outb16, tg):
    """z = P^T D'^{-1} (ams_ap = a^T - s^T + 0); back-transpose.

    ams_ap: PSUM AP already holding a^T - B_sl^T xT (s-MM fused upstream).
    """
    ident32_r, _, _ = consts
    inv_p = prep["inv_p"]

    z = zsb.tile([R, MS], F32R, tag=f"z{tg}", name="z0")
    nc.vector.tensor_scalar(out=z[:], in0=ams_ap, scalar1=EPS,
                            scalar2=inv_p[:], op0=ALU.add, op1=ALU.mult)

    for wk, sign in zip(prep["chain"], (-1.0, 1.0, 1.0, 1.0, 1.0)):
        pz = zps("z")
        nc.tensor.matmul(pz[:], lhsT=wk[:], rhs=z[:], start=True, stop=True)
        zn = zsb.tile([R, MS], F32R, tag=f"z{tg}", name="zn")
        nc.vector.scalar_tensor_tensor(out=zn[:], in0=pz[:], scalar=sign,
                                       in1=z[:], op0=ALU.mult, op1=ALU.add)
        z = zn

    pun = punp.tile([128, MC, R], F32R, tag="pun", name="pun")
    for i in range(MC):
        nc.tensor.transpose(pun[:, i], z[:, i * 128:(i + 1) * 128],
                            ident32_r)
    nc.scalar.copy(out32[:], pun[:])
    if outb16 is not None:
        nc.vector.tensor_copy(outb16[:], pun[:])


def _build():
    nc = bacc.Bacc("TRN2", target_bir_lowering=False, debug=False,
                   num_devices=NCORES)

    x_my = nc.dram_tensor("x_my", [B, MS, N], F32, kind="ExternalInput").ap()
    u_my = nc.dram_tensor("u_my", [B, MS, R], F32, kind="ExternalInput").ap()
    v_full = nc.dram_tensor("v_full", [B, N, R], F32,
                            kind="ExternalInput").ap()
    v_my = nc.dram_tensor("v_my", [B, MS, R], F32, kind="ExternalInput").ap()
    u_out = nc.dram_tensor("u_out", [B, MS, R], F32,
                           kind="ExternalOutput").ap()
    v_out = nc.dram_tensor("v_out", [B, MS, R], F32,
                           kind="ExternalOutput").ap()

    rs_ins = [nc.dram_tensor(f"rs_in_{b}", [NCORES * R, 512 + R], F32)
              for b in range(B)]
    rs_outs = [nc.dram_tensor(f"rs_out_{b}", [R, 512 + R], F32)
               for b in range(B)]

    with tile.TileContext(nc) as tc, ExitStack() as ctx:
        const = ctx.enter_context(tc.tile_pool(name="const", bufs=1))
        xbp = ctx.enter_context(tc.tile_pool(name="xbp", bufs=1))
        xgp = ctx.enter_context(tc.tile_pool(name="xgp", bufs=3))
        xtp = ctx.enter_context(tc.tile_pool(name="xtp", bufs=4))
        vp = ctx.enter_context(tc.tile_pool(name="vp", bufs=2))
        smp = ctx.enter_context(tc.tile_pool(name="smp", bufs=2))
        zsb = ctx.enter_context(tc.tile_pool(name="zsb", bufs=3))
        a2sp = ctx.enter_context(tc.tile_pool(name="a2sp", bufs=2))
        # PSUM banks: ppt 3 + pa1 1 + pzu 1 + pw 1 + pun 1 + pa2 1 = 8
        ppt = ctx.enter_context(tc.tile_pool(name="ppt", bufs=3,
                                             space="PSUM"))
        pa1p = ctx.enter_context(tc.tile_pool(name="pa1", bufs=1,
                                              space="PSUM"))
        pzup = ctx.enter_context(tc.tile_pool(name="pzu", bufs=1,
                                              space="PSUM"))
        pwp = ctx.enter_context(tc.tile_pool(name="pw", bufs=1,
                                             space="PSUM"))
        punp = ctx.enter_context(tc.tile_pool(name="pun", bufs=1,
                                              space="PSUM"))
        pa2p = ctx.enter_context(tc.tile_pool(name="pa2", bufs=1,
                                              space="PSUM"))

        ident128_b = const.tile([128, 128], BF16)
        make_identity(nc, ident128_b)
        ident128_f = const.tile([128, 128], F32)
        make_identity(nc, ident128_f)
        ident32_f = const.tile([R, R], F32)
        make_identity(nc, ident32_f)
        masksl_f = const.tile([R, R], F32)
        make_lower_triangular(nc, masksl_f, val=1.0, diag=False)
        ident32_r = const.tile([R, R], F32R)
        nc.vector.tensor_copy(ident32_r[:], ident32_f[:])
        masksl_r = const.tile([R, R], F32R)
        nc.vector.tensor_copy(masksl_r[:], masksl_f[:])
        consts = (ident32_r[:], masksl_r[:], ident32_r[:])

        def zps(nm):
            return pzup.tile([R, MS], F32, tag="zu", name=nm)

        def zps_v(nm):
            return pa2p.tile([128, MS], F32, tag="pa2", name=nm)[:R, :]

        # ---------- hoisted: v loads + casts; preps spread over phase1(0) --
        vbs = []
        u_preps = {}
        for b in range(B):
            v32 = vp.tile([128, NCH, R], F32, tag="v32", name="v32",
                          bufs=4)
            nc.sync.dma_start(v32[:],
                              v_full[b].rearrange("(c p) r -> p c r", p=128))
            vb = vp.tile([128, NCH, R], BF16, tag=f"vb{b}", name="vb",
                         bufs=1)
            nc.vector.tensor_copy(vb[:], v32[:])
            vbs.append(vb)

        def emit_prep(b):
            pb1 = zps("pb1")
            for j in range(NCH):
                nc.tensor.matmul(pb1[:, :R], lhsT=vbs[b][:, j, :],
                                 rhs=vbs[b][:, j, :], start=(j == 0),
                                 stop=(j == NCH - 1), skip_group_check=True)
            b1_sb = smp.tile([R, R], F32R, tag=f"b1s{b}", name="b1_sb",
                             bufs=1)
            nc.scalar.copy(b1_sb[:], pb1[:, :R])
            u_preps[b] = _gram_prep(nc, smp, pwp, punp, consts, b1_sb,
                                    f"u{b}")

        state = {}

        def emit_solve_v(b):
            st = state[b]
            a2t = smp.tile([R, MS], F32R, tag="a2t", name="a2t")
            nc.sync.dma_start(a2t[:],
                              rs_outs[b].ap()[:, 0:512].bitcast(F32R))
            b2_sb = smp.tile([R, R], F32R, tag="b2s", name="b2_sb")
            nc.sync.dma_start(b2_sb[:],
                              rs_outs[b].ap()[:, 512:512 + R].bitcast(F32R))
            prep = _gram_prep(nc, smp, pwp, punp, consts, b2_sb, f"v{b}")
            vn32 = smp.tile([128, MC, R], F32, tag="vn32", name="vn32")
            ps = zps_v("s")
            nc.tensor.matmul(ps[:], lhsT=prep["nbsl"][:], rhs=st["vT"][:],
                             start=True, stop=True)
            ams = smp.tile([R, MS], F32R, tag="at2", name="ams")
            nc.vector.tensor_tensor(out=ams[:], in0=ps[:], in1=a2t[:],
                                    op=ALU.add)
            _apply_solve(nc, smp, zsb, punp, zps_v, consts, prep,
                         ams[:], vn32, None, "v")
            nc.sync.dma_start(v_out[b].rearrange("(i p) r -> p i r", p=128),
                              vn32[:])

        p2state = {}

        def emit_phase2_part(bp, part):
            # one i-step quartet of col-tiled a2T MMs for batch bp
            gp, i = part // MC, part % MC
            unb_p, xb_p = p2state[bp]
            if i == 0 and f"pa2_{bp}_{gp}" not in p2state:
                p2state[f"pa2_{bp}_{gp}"] = pa2p.tile(
                    [128, MS], F32, tag="pa2", name="pa2")
            pa2 = p2state[f"pa2_{bp}_{gp}"]
            for p in range(4):
                g2 = gp * 4 + p
                nc.tensor.matmul(
                    pa2[p * R:(p + 1) * R, :], lhsT=unb_p[:, i, :],
                    rhs=xb_p[:, i, g2 * 512:(g2 + 1) * 512],
                    start=(i == 0), stop=(i == MC - 1),
                    tile_position=(0, p * R), skip_group_check=True)
            if i == MC - 1:
                a2st = a2sp.tile([128, MS], F32, tag="a2st", name="a2st")
                nc.scalar.copy(a2st[:], pa2[:])
                nc.sync.dma_start(
                    rs_ins[bp].ap()[ds(gp * 4 * R, 4 * R), 0:512], a2st[:])

        def finish_phase2(bp):
            unb_p, _ = p2state[bp]
            pb2 = pwp.tile([R, R], F32, tag="pw", name="pb2")
            for i in range(MC):
                nc.tensor.matmul(pb2[:], lhsT=unb_p[:, i, :],
                                 rhs=unb_p[:, i, :], start=(i == 0),
                                 stop=(i == MC - 1), skip_group_check=True)
            b2st = a2sp.tile([R, R], F32, tag="b2st", name="b2st")
            nc.scalar.copy(b2st[:], pb2[:])
            for c in range(NCORES):
                nc.sync.dma_start(
                    rs_ins[bp].ap()[ds(c * R, R), 512:512 + R], b2st[:])
            nc.gpsimd.collective_compute(
                "ReduceScatter", ALU.add,
                replica_groups=[list(range(NCORES))],
                ins=[rs_ins[bp].ap()], outs=[rs_outs[bp].ap()])

        pref = {}
        NPREF = 2
        for b in range(B):
            bi = b % 2
            # ---------------- per-batch loads + uT/vT ----------------
            u32 = vp.tile([128, MC, R], F32, tag="u32", name="u32")
            nc.sync.dma_start(u32[:],
                              u_my[b].rearrange("(i p) r -> p i r", p=128))
            vm32 = vp.tile([128, MC, R], F32, tag="vm32", name="vm32")
            nc.sync.dma_start(vm32[:],
                              v_my[b].rearrange("(i p) r -> p i r", p=128))

            put = zps("put")
            for i in range(MC):
                nc.tensor.transpose(put[:, i * 128:(i + 1) * 128],
                                    u32[:, i, :], ident128_f[:])
            uT = smp.tile([R, MS], F32R, tag="uT", name="uT")
            nc.scalar.copy(uT[:], put[:])
            pvt = zps("pvt")
            for i in range(MC):
                nc.tensor.transpose(pvt[:, i * 128:(i + 1) * 128],
                                    vm32[:, i, :], ident128_f[:])
            vT = smp.tile([R, MS], F32R, tag="vT", name="vT", bufs=3)
            nc.scalar.copy(vT[:], pvt[:])

            # ---------------- phase 1: stream x ----------------
            if b in pref:
                xb_t = pref.pop(b)
            else:
                xb_t = xbp.tile([128, MC, N], BF16, tag=f"xb{bi}",
                                name="xb")
            pa1 = pa1p.tile([R, MS], F32, tag="pa1", name="pa1")
            x_re = x_my[b].rearrange("(i p) n -> p i n", p=128)
            for g in range(NG):
                if not (b > 0 and g < NPREF):
                    xg = xgp.tile([128, MC, 512], F32, tag="xg", name="xg")
                    nc.sync.dma_start(xg[:],
                                      x_re[:, :, g * 512:(g + 1) * 512])
                    nc.vector.tensor_copy(
                        xb_t[:, :, g * 512:(g + 1) * 512], xg[:])
                for j2 in range(4):
                    j = 4 * g + j2
                    pt = ppt.tile([128, MC, 128], BF16, tag="pt", name="pt")
                    for i in range(MC):
                        nc.tensor.transpose(
                            pt[:, i], xb_t[:, i, j * 128:(j + 1) * 128],
                            ident128_b[:])
                    xt = xtp.tile([128, MC, 128], BF16, tag="xt", name="xt")
                    nc.scalar.copy(xt[:], pt[:])
                    nc.tensor.matmul(pa1[:], lhsT=vbs[b][:, j, :],
                                     rhs=xt.rearrange("p a b -> p (a b)"),
                                     start=(j == 0), stop=False,
                                     skip_group_check=True)
                if b == 0 and g < B:
                    emit_prep(g)
                if b >= 1 and g < 4:
                    emit_phase2_part(b - 1, 2 * g)
                    emit_phase2_part(b - 1, 2 * g + 1)
            if b >= 1:
                finish_phase2(b - 1)
                if b >= 3:
                    emit_solve_v(b - 3)

            # prefetch next batch's first groups so the x stream never stalls
            if b + 1 < B:
                xb_n = xbp.tile([128, MC, N], BF16, tag=f"xb{(b + 1) % 2}",
                                name="xbn")
                x_re_n = x_my[b + 1].rearrange("(i p) n -> p i n", p=128)
                for g in range(NPREF):
                    xg = xgp.tile([128, MC, 512], F32, tag="xg", name="xg")
                    nc.sync.dma_start(
                        xg[:], x_re_n[:, :, g * 512:(g + 1) * 512])
                    nc.vector.tensor_copy(
                        xb_n[:, :, g * 512:(g + 1) * 512], xg[:])
                pref[b + 1] = xb_n

            # ---------------- u solve (s-MM fused into pa1 group) --------
            nc.tensor.matmul(pa1[:], lhsT=u_preps[b]["nbsl"][:], rhs=uT[:],
                             start=False, stop=True, skip_group_check=True)
            un32 = smp.tile([128, MC, R], F32, tag="un32", name="un32")
            unb = smp.tile([128, MC, R], BF16, tag="unb", name="unb")
            _apply_solve(nc, smp, zsb, punp, zps, consts, u_preps[b],
                         pa1[:], un32, unb, "u")
            nc.sync.dma_start(u_out[b].rearrange("(i p) r -> p i r", p=128),
                              un32[:])

            p2state[b] = (unb, xb_t)
            state[b] = {"vT": vT}

        # last batch: dense phase 2 + RS, then remaining v-solves
        # b2 first so its replicate DMAs overlap the pack matmuls
        unb_t, _ = p2state[B - 1]
        pb2t = pwp.tile([R, R], F32, tag="pw", name="pb2t")
        for i in range(MC):
            nc.tensor.matmul(pb2t[:], lhsT=unb_t[:, i, :],
                             rhs=unb_t[:, i, :], start=(i == 0),
                             stop=(i == MC - 1), skip_group_check=True)
        b2stt = a2sp.tile([R, R], F32, tag="b2st", name="b2stt")
        nc.scalar.copy(b2stt[:], pb2t[:])
        for c in range(NCORES):
            nc.sync.dma_start(
                rs_ins[B - 1].ap()[ds(c * R, R), 512:512 + R], b2stt[:])
        # second pack borrows the zu bank so the two packs don't serialize
        p2state[f"pa2_{B - 1}_1"] = pzup.tile([128, MS], F32, tag="zu",
                                              name="pa2z")
        for part in range(NG):
            emit_phase2_part(B - 1, part)
        nc.gpsimd.collective_compute(
            "ReduceScatter", ALU.add,
            replica_groups=[list(range(NCORES))],
            ins=[rs_ins[B - 1].ap()], outs=[rs_outs[B - 1].ap()])
        emit_solve_v(B - 3)
        emit_solve_v(B - 2)
        emit_solve_v(B - 1)

    nc.compile()
    return nc


def kernel(x, u, v):
    global LAST_RESULT
    if "nc" not in _CACHE:
        _CACHE["nc"] = _build()
    nc = _CACHE["nc"]

    x = np.ascontiguousarray(x, dtype=np.float32)
    u = np.ascontiguousarray(u, dtype=np.float32)
    v = np.ascontiguousarray(v, dtype=np.float32)

    in_maps = []
    for c in range(NCORES):
        sl = slice(c * MS, (c + 1) * MS)
        in_maps.append({
            "x_my": np.ascontiguousarray(x[:, sl, :]),
            "u_my": np.ascontiguousarray(u[:, sl, :]),
            "v_full": v,
            "v_my": np.ascontiguousarray(v[:, sl, :]),
        })

    res = run_bass_kernel_spmd(nc, in_maps, list(range(NCORES)),
                               trace=os.environ.get("KBENCH_TRACE") == "1")
    LAST_RESULT = res
    u_new = np.concatenate([res.results[c]["u_out"] for c in range(NCORES)],
                           axis=1)
    v_new = np.concatenate([res.results[c]["v_out"] for c in range(NCORES)],
                           axis=1)
    return (u_new, v_new)
